# revision 10
# baseline (speedup 1.0000x reference)
"""Trainium2 Bass kernel for nn_ObjectLoss (YOLO-style objectness BCE loss).

Reference semantics (per scale s with grid G):
    pred = out_s[..., 4]                            # objectness channel
    per-target best anchor by IoU of (w,h) boxes; cells (b, a*, ty*G, tx*G)
    with iou > 0.5 get gt=1 (idempotent scatter)
    loss_s = mean(-(gt*log(p) + (1-gt)*log1p(-p)))
    loss = sum over 3 scales

Strategy (8 cores, data-parallel over batch, 2 batches/core):
  - Only channel 4 of 85 is ever needed: gather it with strided DMA
    (1/85th of the bytes).  The gather is descriptor-generation bound
    (~32k single-element descriptors/core), so the chunks are split
    across the independent descriptor generators: SP-HWDGE (nc.sync),
    SWDGE (nc.gpsimd) and optionally ACT-HWDGE (nc.scalar), all of
    whose DMA_DIRECT2D gen phases run concurrently.
  - All gather gens are issued up-front (before any compute) so every
    ring starts generating as soon as the preamble ends.
  - gt grid built on-device without scatter: one-hot(row) x one-hot(col)
    outer products accumulated over targets == a small matmul per batch.
  - BCE = -sum(L1) + sum(gt*(L1-L2)) with L1=ln(1-p), L2=ln(p).  The
    L1 sums are accumulated on the Vector engine (DVE accumulator) to
    keep the Scalar/ACT queue short; per-(chunk) partial sums land in a
    [128, 2*NT] tile DMA'd out raw and reduced on host.

Hardware note: each compute instruction can encode only ONE semaphore
wait, so the program is shaped to give every instruction at most one
unobserved cross-engine dependency: all small inputs ride in a single
"consts" DMA, each engine touches it early (the ACT warm-up copy is
placed AFTER the ACT-ring DMA gens but before the activations), and
psum-consuming ops are split so they wait only on the PE semaphore.
"""

import os
import sys

import numpy as np

for _p in ("/opt/trn_rl_repo", "/root/.axon_site/_ro/trn_rl_repo"):
    if os.path.isdir(_p) and _p not in sys.path:
        sys.path.insert(0, _p)
        break

GS = (64, 32, 16)  # grid size per scale (H == W)
B, A, T, C = 16, 3, 64, 85
NCORES = 8
BL = B // NCORES  # batches per core
OBJ = 4  # objectness channel

# pred/gt layout: one chunk = one (scale, batch) pair = one gather DMA =
# one gt psum group.  n = A*g rows; rows are J-packed into P = n/J
# partitions (row = p*J + j, free dim = (j, w)) so big scales stay a
# single DMA (amortizing the ~590ns SWDGE per-DMA fixed cost) while
# matmul output tiles keep <=128 partitions.
CHUNKS = [
    (0, 0, 0, 192),  # scale 0 (g=64), batch 0: [96, 2*64]
    (0, 1, 0, 192),
    (1, 0, 0, 96),   # scale 1 (g=32): [96, 32]
    (1, 1, 0, 96),
    (2, 0, 0, 48),   # scale 2 (g=16): [48, 16]
    (2, 1, 0, 48),
]
J_BY_SCALE = {0: 2, 1: 1, 2: 1}
NT = len(CHUNKS)

# Measured (HW traces): every gather path converges to ~1.4 random HBM
# reads/ns per core, so single-element descriptors floor at ~22us.
# Paired descriptors (one contiguous 344B run covering the objectness
# of w and w+1 -> 172B/cell) halve the descriptor count and shift the
# cost toward HBM bandwidth (~15-18us floor).  Two concurrently active
# SDMA queues measurably slow each other, so ALL gathers ride SWDGE
# (gpsimd, aggregated packets) while consts/output use the otherwise
# idle SP ring.
# (ring, chunk); issue order within a ring = order in this list.
ISSUE_PLAN = [
    ("gpsimd", 4),  # s2b0 first: lands early, starts the ACT pipeline
    ("gpsimd", 0),  # s0b0 (6144 paired descs)
    ("gpsimd", 2),  # s1b0
    ("gpsimd", 1),  # s0b1
    ("gpsimd", 3),  # s1b1
    ("gpsimd", 5),  # s2b1
]

# consts layout [128, NCONST]: per-scale iota repeated 4x, anchors
# (replicated across partitions), targets re-laid-out as [t, (b k)],
# a ones column and a zeros column (activation bias operands).
IOTA_OFF = []
_off = 0
for _g in GS:
    IOTA_OFF.append(_off)
    _off += 4 * _g
ANC_OFF = _off          # 18 cols: (s, a, d)
TGT_OFF = _off + 18     # 10 cols: (b, k), rows = t
ONE_OFF = TGT_OFF + 10  # 1.0
ZERO_OFF = ONE_OFF + 1  # 0.0
NCONST = ZERO_OFF + 1

_CONST_BASE = None


def _const_base():
    global _CONST_BASE
    if _CONST_BASE is None:
        c = np.zeros((128, NCONST), np.float32)
        for s, g in enumerate(GS):
            c[:, IOTA_OFF[s] : IOTA_OFF[s] + 4 * g] = np.tile(
                np.arange(g, dtype=np.float32), 4
            )[None, :]
        c[:, ONE_OFF] = 1.0
        _CONST_BASE = c
    return _CONST_BASE


_BUILT = None


def _build():
    """Build the SPMD bass program (same program on all 8 cores)."""
    global _BUILT
    if _BUILT is not None:
        return _BUILT

    from contextlib import ExitStack

    import concourse.bass as bass
    import concourse.tile as tile
    from concourse import mybir

    f32 = mybir.dt.float32
    Alu = mybir.AluOpType
    Act = mybir.ActivationFunctionType

    nc = bass.Bass()
    d_outs = [
        nc.declare_dram_parameter(f"out{s}", [BL, A, g, g, C], f32, isOutput=False)
        for s, g in enumerate(GS)
    ]
    d_const = nc.declare_dram_parameter("consts", [128, NCONST], f32, isOutput=False)
    d_part = nc.declare_dram_parameter("partial", [128, NT], f32, isOutput=True)

    with tile.TileContext(nc) as tc, ExitStack() as ctx:
        sb = ctx.enter_context(tc.tile_pool(name="sb", bufs=1))
        ps = ctx.enter_context(tc.tile_pool(name="ps", bufs=4, space="PSUM"))

        # ---------- the single small-input load (SP ring; keeps SWDGE free) --
        consts = sb.tile([128, NCONST], f32, tag="consts")
        nc.sync.dma_start(out=consts[:], in_=d_const[:])

        # ---------- all gather DMAs up-front (single-element descs) ------
        eng = {"sync": nc.sync, "scalar": nc.scalar, "gpsimd": nc.gpsimd}
        pred_tiles = {}  # k -> [P, J*g] tile, row (p*J+j) at free (j, w)
        for ring, k in ISSUE_PLAN:
            s, b, r0, n = CHUNKS[k]
            g = GS[s]
            J = J_BY_SCALE[s]
            P = n // J
            gr0 = b * A * g + r0
            pr = sb.tile([P, J * g], f32, tag=f"pred{k}")
            src = d_outs[s][:].rearrange("b a h w c -> (b a h) w c")[
                gr0 : gr0 + n, :, OBJ : OBJ + 1
            ].rearrange("(p j) w one -> p j (w one)", j=J)
            with nc.allow_non_contiguous_dma("objectness channel gather"):
                eng[ring].dma_start(
                    out=pr[:].rearrange("p (j w) -> p j w", j=J), in_=src
                )
            pred_tiles[k] = pr

        # ACT warm-up touch of consts so later activations never need a
        # consts wait (one sem wait max per instruction).  Placed after the
        # ACT-ring DMA gens so those start immediately.
        warm = sb.tile([1, 1], f32, tag="warm")
        nc.scalar.copy(warm[:], consts[0:1, 0:1])

        ancb = consts[0:64, ANC_OFF : ANC_OFF + 18]  # (s, a, d)
        tgt = consts[0:64, TGT_OFF : TGT_OFF + 10]  # rows=t, cols=(b, k)

        # ---------- per-target math (all [64, *] tiles; partition = t) ----------
        tgt_kb = tgt.rearrange("p (b k) -> p k b", b=BL)  # [64, 5, BL]
        xsel = tgt_kb[:, 1:3, :]  # (tx, ty) per b
        wsel = tgt_kb[:, 3:5, :]  # (tw, th) per b

        x4 = sb.tile([64, 12], f32, tag="x4")  # (s, dir, b): x*G
        x4m1 = sb.tile([64, 12], f32, tag="x4m1")  # x*G - 1
        twth = sb.tile([64, 12], f32, tag="twth")  # (s, d, b): box wh in grid units
        for s, g in enumerate(GS):
            o = x4[:, 4 * s : 4 * s + 4].rearrange("p (k b) -> p k b", k=2)
            nc.vector.tensor_scalar(
                out=o, in0=xsel, scalar1=float(g), scalar2=None, op0=Alu.mult
            )
            o = x4m1[:, 4 * s : 4 * s + 4].rearrange("p (k b) -> p k b", k=2)
            nc.vector.tensor_scalar(
                out=o,
                in0=xsel,
                scalar1=float(g),
                scalar2=1.0,
                op0=Alu.mult,
                op1=Alu.subtract,
            )
            o = twth[:, 4 * s : 4 * s + 4].rearrange("p (k b) -> p k b", k=2)
            nc.vector.tensor_scalar(
                out=o, in0=wsel, scalar1=float(g), scalar2=None, op0=Alu.mult
            )

        # ---------- one-hot row/col masks ----------
        # m4[s][t, (dir, b, i)] = 1 iff floor(x_dirb * G) == i, via
        # (iota <= x) * (iota > x-1); x = coord*G is exact (G power of two)
        m4 = []
        for s, g in enumerate(GS):
            io = consts[0:64, IOTA_OFF[s] : IOTA_OFF[s] + 4 * g].rearrange(
                "p (k g) -> p k g", k=4
            )
            xb = x4[:, 4 * s : 4 * s + 4][:, :, None].broadcast_to([64, 4, g])
            xm1b = x4m1[:, 4 * s : 4 * s + 4][:, :, None].broadcast_to([64, 4, g])
            at = sb.tile([64, 4 * g], f32, tag=f"onehA{s}")
            bt = sb.tile([64, 4 * g], f32, tag=f"onehB{s}")
            mt = sb.tile([64, 4 * g], f32, tag=f"m4_{s}")
            atr = at[:].rearrange("p (k g) -> p k g", k=4)
            btr = bt[:].rearrange("p (k g) -> p k g", k=4)
            nc.vector.tensor_tensor(out=atr, in0=io, in1=xb, op=Alu.is_le)
            nc.vector.tensor_tensor(out=btr, in0=io, in1=xm1b, op=Alu.is_gt)
            nc.vector.tensor_tensor(out=mt[:], in0=at[:], in1=bt[:], op=Alu.mult)
            m4.append(mt)

        # ---------- IoU / best-anchor (free layout (s, a, b) = [64, 18]) ----------
        def r3(t):  # [64,18] -> [64,3,3,2]
            return t[:].rearrange("p (s a b) -> p s a b", s=3, a=3)

        twth_r = twth[:].rearrange("p (s d b) -> p s d b", s=3, d=2)
        anc_r = ancb.rearrange("p (s a d) -> p s a d", s=3, a=3)
        tw_b = twth_r[:, :, 0, :][:, :, None, :].broadcast_to([64, 3, 3, 2])
        th_b = twth_r[:, :, 1, :][:, :, None, :].broadcast_to([64, 3, 3, 2])
        aw_b = anc_r[:, :, :, 0][:, :, :, None].broadcast_to([64, 3, 3, 2])
        ah_b = anc_r[:, :, :, 1][:, :, :, None].broadcast_to([64, 3, 3, 2])

        m1 = sb.tile([64, 18], f32, tag="m1")
        m2 = sb.tile([64, 18], f32, tag="m2")
        inter = sb.tile([64, 18], f32, tag="inter")
        nc.vector.tensor_tensor(out=r3(m1), in0=tw_b, in1=aw_b, op=Alu.min)
        nc.vector.tensor_tensor(out=r3(m2), in0=th_b, in1=ah_b, op=Alu.min)
        nc.vector.tensor_tensor(out=inter[:], in0=m1[:], in1=m2[:], op=Alu.mult)

        areat = sb.tile([64, 6], f32, tag="areat")  # (s, b) = tw*th
        nc.vector.tensor_tensor(
            out=areat[:].rearrange("p (s b) -> p s b", s=3),
            in0=twth_r[:, :, 0, :],
            in1=twth_r[:, :, 1, :],
            op=Alu.mult,
        )
        areaa = sb.tile([64, 9], f32, tag="areaa")  # (s, a) = aw*ah
        nc.vector.tensor_tensor(
            out=areaa[:].rearrange("p (s a) -> p s a", s=3),
            in0=anc_r[:, :, :, 0],
            in1=anc_r[:, :, :, 1],
            op=Alu.mult,
        )

        union = sb.tile([64, 18], f32, tag="union")
        areaa_b = (
            areaa[:]
            .rearrange("p (s a) -> p s a", s=3)[:, :, :, None]
            .broadcast_to([64, 3, 3, 2])
        )
        areat_b = (
            areat[:]
            .rearrange("p (s b) -> p s b", s=3)[:, :, None, :]
            .broadcast_to([64, 3, 3, 2])
        )
        nc.vector.tensor_tensor(out=r3(union), in0=areaa_b, in1=areat_b, op=Alu.add)
        nc.vector.tensor_tensor(
            out=union[:], in0=union[:], in1=inter[:], op=Alu.subtract
        )

        # iou > 0.5  <=>  2*inter > union   (division-free)
        cmp2 = sb.tile([64, 18], f32, tag="cmp2")
        nc.vector.scalar_tensor_tensor(
            out=cmp2[:],
            in0=inter[:],
            scalar=2.0,
            in1=union[:],
            op0=Alu.mult,
            op1=Alu.is_gt,
        )

        # argmax over anchors via cross products (iou_a >= iou_b <=>
        # inter_a*union_b >= inter_b*union_a); first-wins tie-breaking
        inter_r = r3(inter)
        union_r = r3(union)

        def pairprod(name, ia, ib):
            t = sb.tile([64, 6], f32, tag=name)
            nc.vector.tensor_tensor(
                out=t[:].rearrange("p (s b) -> p s b", s=3),
                in0=inter_r[:, :, ia, :],
                in1=union_r[:, :, ib, :],
                op=Alu.mult,
            )
            return t

        p01 = pairprod("p01", 0, 1)
        p10 = pairprod("p10", 1, 0)
        p02 = pairprod("p02", 0, 2)
        p20 = pairprod("p20", 2, 0)
        p12 = pairprod("p12", 1, 2)
        p21 = pairprod("p21", 2, 1)
        ge01 = sb.tile([64, 6], f32, tag="ge01")
        ge02 = sb.tile([64, 6], f32, tag="ge02")
        ge12 = sb.tile([64, 6], f32, tag="ge12")
        nc.vector.tensor_tensor(out=ge01[:], in0=p01[:], in1=p10[:], op=Alu.is_ge)
        nc.vector.tensor_tensor(out=ge02[:], in0=p02[:], in1=p20[:], op=Alu.is_ge)
        nc.vector.tensor_tensor(out=ge12[:], in0=p12[:], in1=p21[:], op=Alu.is_ge)

        oht = sb.tile([64, 18], f32, tag="oht")
        oht_r = r3(oht)
        # oh0 = ge01 & ge02
        nc.vector.tensor_tensor(
            out=oht_r[:, :, 0, :],
            in0=ge01[:].rearrange("p (s b) -> p s b", s=3),
            in1=ge02[:].rearrange("p (s b) -> p s b", s=3),
            op=Alu.mult,
        )
        # oh1 = (1 - ge01) & ge12
        n01 = sb.tile([64, 6], f32, tag="n01")
        nc.vector.tensor_scalar(
            out=n01[:],
            in0=ge01[:],
            scalar1=-1.0,
            scalar2=1.0,
            op0=Alu.mult,
            op1=Alu.add,
        )
        nc.vector.tensor_tensor(
            out=oht_r[:, :, 1, :],
            in0=n01[:].rearrange("p (s b) -> p s b", s=3),
            in1=ge12[:].rearrange("p (s b) -> p s b", s=3),
            op=Alu.mult,
        )
        # oh2 = 1 - oh0 - oh1  (oh0, oh1 mutually exclusive)
        s01 = sb.tile([64, 6], f32, tag="s01")
        nc.vector.tensor_tensor(
            out=s01[:].rearrange("p (s b) -> p s b", s=3),
            in0=oht_r[:, :, 0, :],
            in1=oht_r[:, :, 1, :],
            op=Alu.add,
        )
        nc.vector.tensor_scalar(
            out=oht_r[:, :, 2, :],
            in0=s01[:].rearrange("p (s b) -> p s b", s=3),
            scalar1=-1.0,
            scalar2=1.0,
            op0=Alu.mult,
            op1=Alu.add,
        )

        # w4 = onehot(best anchor) & (iou > 0.5)
        w4 = sb.tile([64, 18], f32, tag="w4")
        nc.vector.tensor_tensor(out=w4[:], in0=oht[:], in1=cmp2[:], op=Alu.mult)

        # ---------- Mja = one-hot(j) replicated per anchor, weighted ----------
        mja = []  # [s][b] -> [64, 3*g] tile, cols (a, h)
        for s, g in enumerate(GS):
            row = []
            for b in range(BL):
                t = sb.tile([64, 3 * g], f32, tag=f"mja{s}_{b}")
                mj_sb = m4[s][:, (2 + b) * g : (3 + b) * g][:, None, :].broadcast_to(
                    [64, 3, g]
                )
                wv = r3(w4)[:, s, :, b][:, :, None].broadcast_to([64, 3, g])
                nc.vector.tensor_tensor(
                    out=t[:].rearrange("p (a g) -> p a g", a=3),
                    in0=mj_sb,
                    in1=wv,
                    op=Alu.mult,
                )
                row.append(t)
            mja.append(row)

        # ---------- per-chunk: gt matmul, BCE ----------
        acc = sb.tile([128, NT], f32, tag="acc")
        nc.vector.memset(acc[:], 0.0)

        for _, k in ISSUE_PLAN:  # process in (approx) arrival order
            s, b, r0, n = CHUNKS[k]
            g = GS[s]
            J = J_BY_SCALE[s]
            P = n // J
            pr_ap = pred_tiles[k][:]

            # gt counts: J matmuls; matmul j covers rows p*J+j -> psum
            # [P, g], binarized into the matching (j, w) slab of gtb
            gtb = sb.tile([P, J * g], f32, tag=f"gtb{k}")
            for j in range(J):
                pt = ps.tile([P, g], f32, tag="gt")
                wj = (
                    mja[s][b][:]
                    .rearrange("t (p j) -> t j p", j=J)[:, j, :]
                )
                nc.tensor.matmul(
                    pt[:],
                    wj,
                    m4[s][:, b * g : (b + 1) * g],
                    start=True,
                    stop=True,
                )
                # binarize gt counts (sole op waiting on PE)
                nc.vector.tensor_scalar(
                    out=gtb[:, j * g : (j + 1) * g],
                    in0=pt[:],
                    scalar1=0.5,
                    scalar2=None,
                    op0=Alu.is_ge,
                )

            # BCE pieces: L1 = ln(1-p), L2 = ln(p)
            l1 = sb.tile([P, J * g], f32, tag=f"l1_{k}")
            l2 = sb.tile([P, J * g], f32, tag=f"l2_{k}")
            dd = sb.tile([P, J * g], f32, tag=f"dd{k}")
            mm = sb.tile([P, J * g], f32, tag=f"mm{k}")
            ee = sb.tile([P, J * g], f32, tag=f"ee{k}")
            nc.scalar.activation(
                out=l1[:],
                in_=pr_ap,
                func=Act.Ln,
                bias=consts[0:P, ONE_OFF : ONE_OFF + 1],
                scale=-1.0,
            )
            nc.scalar.activation(
                out=l2[:],
                in_=pr_ap,
                func=Act.Ln,
                bias=consts[0:P, ZERO_OFF : ZERO_OFF + 1],
            )
            # e = L1 + gtb*(L2-L1); acc[:,k] = sum(e) = -(chunk BCE sum)
            nc.vector.tensor_tensor(out=dd[:], in0=l2[:], in1=l1[:], op=Alu.subtract)
            nc.vector.tensor_tensor(out=mm[:], in0=gtb[:], in1=dd[:], op=Alu.mult)
            nc.vector.scalar_tensor_tensor(
                out=ee[:],
                in0=mm[:],
                scalar=0.0,
                in1=l1[:],
                op0=Alu.bypass,
                op1=Alu.add,
                accum_out=acc[0:P, k : k + 1],
            )

        # raw partials out; host reduces across partitions/cores
        nc.sync.dma_start(out=d_part[:], in_=acc[:])

    _fixup_tail_drain(nc, mybir)
    _BUILT = nc
    return nc


def _fixup_tail_drain(nc, mybir, out_name="partial"):
    """The kernel-tail drain waits on every outstanding semaphore lane, but
    the ISA allows one sync wait per instruction and this walrus refuses to
    split them.  In this kernel every instruction's effect funnels into the
    final 'partial' output DMA (all DMAs and compute feed it transitively),
    so waiting on that DMA's completion semaphore alone is sufficient."""
    fn = nc.m.functions[0]
    out_sem = None
    for blk in fn.blocks:
        for inst in blk.instructions:
            if type(inst).__name__ == "InstDMACopy":
                outs = inst.outs
                if outs and out_name in str(outs[0]):
                    si = inst.sync_info
                    if si is not None and si.on_update:
                        out_sem = si.on_update[0].id
    assert out_sem is not None, "no output DMA with sem update found"
    for blk in fn.blocks:
        for inst in blk.instructions:
            si = inst.sync_info
            if (
                type(inst).__name__ == "InstDrain"
                and si is not None
                and len(si.on_wait) > 1
            ):
                keep = [w for w in si.on_wait if w.id == out_sem]
                assert len(keep) == 1, (
                    f"tail drain: expected exactly one wait on sem {out_sem}, "
                    f"got {[w.id for w in si.on_wait]}"
                )
                inst.sync_info = mybir.SyncInfo(
                    on_wait=keep, on_update=list(si.on_update)
                )


def _make_in_maps(out0, out1, out2, anchors0, anchors1, anchors2, targets):
    base = _const_base()
    anc_flat = np.concatenate(
        [np.asarray(a, np.float32).reshape(-1) for a in (anchors0, anchors1, anchors2)]
    )  # (s, a, d) = 18
    outs = (out0, out1, out2)
    in_maps = []
    for c in range(NCORES):
        sl = slice(c * BL, (c + 1) * BL)
        consts = base.copy()
        consts[:, ANC_OFF : ANC_OFF + 18] = anc_flat[None, :]
        # targets block: rows = t, cols = (b, k)
        tloc = np.asarray(targets[sl], np.float32)  # [BL, T, 5]
        consts[0:T, TGT_OFF : TGT_OFF + 10] = tloc.transpose(1, 0, 2).reshape(T, -1)
        m = {"consts": consts}
        for s in range(3):
            m[f"out{s}"] = np.ascontiguousarray(outs[s][sl])
        in_maps.append(m)
    return in_maps


def _reduce_partials(partials):
    """partials: list of [128, NT] arrays -> scalar loss (float64 accum)."""
    tot = np.zeros(NT, np.float64)
    for p in partials:
        tot += np.asarray(p, np.float64).reshape(-1, NT).sum(axis=0)
    loss = 0.0
    for k, (s, b, r0, n) in enumerate(CHUNKS):
        g = GS[s]
        denom = B * A * g * g
        loss += -tot[k] / denom
    return np.float32(loss)


def _run_hw(in_maps, trace=False):
    from concourse.bass_utils import run_bass_kernel_spmd

    nc = _build()
    br = run_bass_kernel_spmd(nc, in_maps, list(range(NCORES)), trace=trace)
    return br


def kernel(out0, out1, out2, anchors0, anchors1, anchors2, targets):
    in_maps = _make_in_maps(
        out0, out1, out2, anchors0, anchors1, anchors2, targets
    )
    br = _run_hw(in_maps, trace=False)
    partials = [r["partial"] for r in br.results]
    return np.asarray(_reduce_partials(partials), dtype=np.float32)


# revision 11
# speedup vs baseline: 1.1738x; 1.1738x over previous
"""Trainium2 Bass kernel for nn_ObjectLoss (YOLO-style objectness BCE loss).

Reference semantics (per scale s with grid G):
    pred = out_s[..., 4]                            # objectness channel
    per-target best anchor by IoU of (w,h) boxes; cells (b, a*, ty*G, tx*G)
    with iou > 0.5 get gt=1 (idempotent scatter)
    loss_s = mean(-(gt*log(p) + (1-gt)*log1p(-p)))
    loss = sum over 3 scales

Strategy (8 cores, data-parallel over batch, 2 batches/core):
  - Only channel 4 of 85 is ever needed: gather it with strided DMA
    (1/85th of the bytes).  The gather is descriptor-generation bound
    (~32k single-element descriptors/core), so the chunks are split
    across the independent descriptor generators: SP-HWDGE (nc.sync),
    SWDGE (nc.gpsimd) and optionally ACT-HWDGE (nc.scalar), all of
    whose DMA_DIRECT2D gen phases run concurrently.
  - All gather gens are issued up-front (before any compute) so every
    ring starts generating as soon as the preamble ends.
  - gt grid built on-device without scatter: one-hot(row) x one-hot(col)
    outer products accumulated over targets == a small matmul per batch.
  - BCE = -sum(L1) + sum(gt*(L1-L2)) with L1=ln(1-p), L2=ln(p).  The
    L1 sums are accumulated on the Vector engine (DVE accumulator) to
    keep the Scalar/ACT queue short; per-(chunk) partial sums land in a
    [128, 2*NT] tile DMA'd out raw and reduced on host.

Hardware note: each compute instruction can encode only ONE semaphore
wait, so the program is shaped to give every instruction at most one
unobserved cross-engine dependency: all small inputs ride in a single
"consts" DMA, each engine touches it early (the ACT warm-up copy is
placed AFTER the ACT-ring DMA gens but before the activations), and
psum-consuming ops are split so they wait only on the PE semaphore.
"""

import os
import sys

import numpy as np

for _p in ("/opt/trn_rl_repo", "/root/.axon_site/_ro/trn_rl_repo"):
    if os.path.isdir(_p) and _p not in sys.path:
        sys.path.insert(0, _p)
        break

GS = (64, 32, 16)  # grid size per scale (H == W)
B, A, T, C = 16, 3, 64, 85
NCORES = 8
BL = B // NCORES  # batches per core
OBJ = 4  # objectness channel

# pred/gt layout: one chunk = one (scale, batch) pair = one gather DMA =
# one gt psum group.  n = A*g rows; rows are J-packed into P = n/J
# partitions (row = p*J + j, free dim = (j, w)) so big scales stay a
# single DMA (amortizing the ~590ns SWDGE per-DMA fixed cost) while
# matmul output tiles keep <=128 partitions.
CHUNKS = [
    (0, 0, 0, 192),  # scale 0 (g=64), batch 0: [96, 2*64]
    (0, 1, 0, 192),
    (1, 0, 0, 96),   # scale 1 (g=32): [96, 32]
    (1, 1, 0, 96),
    (2, 0, 0, 48),   # scale 2 (g=16): [48, 16]
    (2, 1, 0, 48),
]
J_BY_SCALE = {0: 2, 1: 1, 2: 1}
NT = len(CHUNKS)

# Measured (HW traces): every gather path converges to ~1.4 random HBM
# reads/ns per core, so single-element descriptors floor at ~22us.
# Paired descriptors (one contiguous 344B run covering the objectness
# of w and w+1 -> 172B/cell) halve the descriptor count and shift the
# cost toward HBM bandwidth (~15-18us floor).  Two concurrently active
# SDMA queues measurably slow each other, so ALL gathers ride SWDGE
# (gpsimd, aggregated packets) while consts/output use the otherwise
# idle SP ring.
# (ring, chunk); issue order within a ring = order in this list.
ISSUE_PLAN = [
    ("sync", 4),  # s2b0 first: lands early, starts the ACT pipeline
    ("sync", 0),
    ("sync", 2),
    ("sync", 1),
    ("sync", 3),
    ("sync", 5),
]

# consts layout [128, NCONST]: per-scale iota repeated 4x, anchors
# (replicated across partitions), targets re-laid-out as [t, (b k)],
# a ones column and a zeros column (activation bias operands).
IOTA_OFF = []
_off = 0
for _g in GS:
    IOTA_OFF.append(_off)
    _off += 4 * _g
ANC_OFF = _off          # 18 cols: (s, a, d)
TGT_OFF = _off + 18     # 10 cols: (b, k), rows = t
ONE_OFF = TGT_OFF + 10  # 1.0
ZERO_OFF = ONE_OFF + 1  # 0.0
NCONST = ZERO_OFF + 1

_CONST_BASE = None


def _const_base():
    global _CONST_BASE
    if _CONST_BASE is None:
        c = np.zeros((128, NCONST), np.float32)
        for s, g in enumerate(GS):
            c[:, IOTA_OFF[s] : IOTA_OFF[s] + 4 * g] = np.tile(
                np.arange(g, dtype=np.float32), 4
            )[None, :]
        c[:, ONE_OFF] = 1.0
        _CONST_BASE = c
    return _CONST_BASE


_BUILT = None


def _build():
    """Build the SPMD bass program (same program on all 8 cores)."""
    global _BUILT
    if _BUILT is not None:
        return _BUILT

    from contextlib import ExitStack

    import concourse.bass as bass
    import concourse.tile as tile
    from concourse import mybir

    f32 = mybir.dt.float32
    Alu = mybir.AluOpType
    Act = mybir.ActivationFunctionType

    nc = bass.Bass()
    d_outs = [
        nc.declare_dram_parameter(f"out{s}", [BL, A, g, g, C], f32, isOutput=False)
        for s, g in enumerate(GS)
    ]
    d_const = nc.declare_dram_parameter("consts", [128, NCONST], f32, isOutput=False)
    d_part = nc.declare_dram_parameter("partial", [128, NT], f32, isOutput=True)

    with tile.TileContext(nc) as tc, ExitStack() as ctx:
        sb = ctx.enter_context(tc.tile_pool(name="sb", bufs=1))
        ps = ctx.enter_context(tc.tile_pool(name="ps", bufs=4, space="PSUM"))

        # ---------- the single small-input load (SP ring; keeps SWDGE free) --
        consts = sb.tile([128, NCONST], f32, tag="consts")
        nc.gpsimd.dma_start(out=consts[:], in_=d_const[:])

        # ---------- all gather DMAs up-front (single-element descs) ------
        eng = {"sync": nc.sync, "scalar": nc.scalar, "gpsimd": nc.gpsimd}
        pred_tiles = {}  # k -> [P, J*g] tile, row (p*J+j) at free (j, w)
        for ring, k in ISSUE_PLAN:
            s, b, r0, n = CHUNKS[k]
            g = GS[s]
            J = J_BY_SCALE[s]
            P = n // J
            gr0 = b * A * g + r0
            pr = sb.tile([P, J * g], f32, tag=f"pred{k}")
            src = d_outs[s][:].rearrange("b a h w c -> (b a h) w c")[
                gr0 : gr0 + n, :, OBJ : OBJ + 1
            ].rearrange("(p j) w one -> p j (w one)", j=J)
            with nc.allow_non_contiguous_dma("objectness channel gather"):
                eng[ring].dma_start(
                    out=pr[:].rearrange("p (j w) -> p j w", j=J), in_=src
                )
            pred_tiles[k] = pr

        # ACT warm-up touch of consts so later activations never need a
        # consts wait (one sem wait max per instruction).  Placed after the
        # ACT-ring DMA gens so those start immediately.
        warm = sb.tile([1, 1], f32, tag="warm")
        nc.scalar.copy(warm[:], consts[0:1, 0:1])

        ancb = consts[0:64, ANC_OFF : ANC_OFF + 18]  # (s, a, d)
        tgt = consts[0:64, TGT_OFF : TGT_OFF + 10]  # rows=t, cols=(b, k)

        # ---------- per-target math (all [64, *] tiles; partition = t) ----------
        tgt_kb = tgt.rearrange("p (b k) -> p k b", b=BL)  # [64, 5, BL]
        xsel = tgt_kb[:, 1:3, :]  # (tx, ty) per b
        wsel = tgt_kb[:, 3:5, :]  # (tw, th) per b

        x4 = sb.tile([64, 12], f32, tag="x4")  # (s, dir, b): x*G
        x4m1 = sb.tile([64, 12], f32, tag="x4m1")  # x*G - 1
        twth = sb.tile([64, 12], f32, tag="twth")  # (s, d, b): box wh in grid units
        for s, g in enumerate(GS):
            o = x4[:, 4 * s : 4 * s + 4].rearrange("p (k b) -> p k b", k=2)
            nc.vector.tensor_scalar(
                out=o, in0=xsel, scalar1=float(g), scalar2=None, op0=Alu.mult
            )
            o = x4m1[:, 4 * s : 4 * s + 4].rearrange("p (k b) -> p k b", k=2)
            nc.vector.tensor_scalar(
                out=o,
                in0=xsel,
                scalar1=float(g),
                scalar2=1.0,
                op0=Alu.mult,
                op1=Alu.subtract,
            )
            o = twth[:, 4 * s : 4 * s + 4].rearrange("p (k b) -> p k b", k=2)
            nc.vector.tensor_scalar(
                out=o, in0=wsel, scalar1=float(g), scalar2=None, op0=Alu.mult
            )

        # ---------- one-hot row/col masks ----------
        # m4[s][t, (dir, b, i)] = 1 iff floor(x_dirb * G) == i, via
        # (iota <= x) * (iota > x-1); x = coord*G is exact (G power of two)
        m4 = []
        for s, g in enumerate(GS):
            io = consts[0:64, IOTA_OFF[s] : IOTA_OFF[s] + 4 * g].rearrange(
                "p (k g) -> p k g", k=4
            )
            xb = x4[:, 4 * s : 4 * s + 4][:, :, None].broadcast_to([64, 4, g])
            xm1b = x4m1[:, 4 * s : 4 * s + 4][:, :, None].broadcast_to([64, 4, g])
            at = sb.tile([64, 4 * g], f32, tag=f"onehA{s}")
            bt = sb.tile([64, 4 * g], f32, tag=f"onehB{s}")
            mt = sb.tile([64, 4 * g], f32, tag=f"m4_{s}")
            atr = at[:].rearrange("p (k g) -> p k g", k=4)
            btr = bt[:].rearrange("p (k g) -> p k g", k=4)
            nc.vector.tensor_tensor(out=atr, in0=io, in1=xb, op=Alu.is_le)
            nc.vector.tensor_tensor(out=btr, in0=io, in1=xm1b, op=Alu.is_gt)
            nc.vector.tensor_tensor(out=mt[:], in0=at[:], in1=bt[:], op=Alu.mult)
            m4.append(mt)

        # ---------- IoU / best-anchor (free layout (s, a, b) = [64, 18]) ----------
        def r3(t):  # [64,18] -> [64,3,3,2]
            return t[:].rearrange("p (s a b) -> p s a b", s=3, a=3)

        twth_r = twth[:].rearrange("p (s d b) -> p s d b", s=3, d=2)
        anc_r = ancb.rearrange("p (s a d) -> p s a d", s=3, a=3)
        tw_b = twth_r[:, :, 0, :][:, :, None, :].broadcast_to([64, 3, 3, 2])
        th_b = twth_r[:, :, 1, :][:, :, None, :].broadcast_to([64, 3, 3, 2])
        aw_b = anc_r[:, :, :, 0][:, :, :, None].broadcast_to([64, 3, 3, 2])
        ah_b = anc_r[:, :, :, 1][:, :, :, None].broadcast_to([64, 3, 3, 2])

        m1 = sb.tile([64, 18], f32, tag="m1")
        m2 = sb.tile([64, 18], f32, tag="m2")
        inter = sb.tile([64, 18], f32, tag="inter")
        nc.vector.tensor_tensor(out=r3(m1), in0=tw_b, in1=aw_b, op=Alu.min)
        nc.vector.tensor_tensor(out=r3(m2), in0=th_b, in1=ah_b, op=Alu.min)
        nc.vector.tensor_tensor(out=inter[:], in0=m1[:], in1=m2[:], op=Alu.mult)

        areat = sb.tile([64, 6], f32, tag="areat")  # (s, b) = tw*th
        nc.vector.tensor_tensor(
            out=areat[:].rearrange("p (s b) -> p s b", s=3),
            in0=twth_r[:, :, 0, :],
            in1=twth_r[:, :, 1, :],
            op=Alu.mult,
        )
        areaa = sb.tile([64, 9], f32, tag="areaa")  # (s, a) = aw*ah
        nc.vector.tensor_tensor(
            out=areaa[:].rearrange("p (s a) -> p s a", s=3),
            in0=anc_r[:, :, :, 0],
            in1=anc_r[:, :, :, 1],
            op=Alu.mult,
        )

        union = sb.tile([64, 18], f32, tag="union")
        areaa_b = (
            areaa[:]
            .rearrange("p (s a) -> p s a", s=3)[:, :, :, None]
            .broadcast_to([64, 3, 3, 2])
        )
        areat_b = (
            areat[:]
            .rearrange("p (s b) -> p s b", s=3)[:, :, None, :]
            .broadcast_to([64, 3, 3, 2])
        )
        nc.vector.tensor_tensor(out=r3(union), in0=areaa_b, in1=areat_b, op=Alu.add)
        nc.vector.tensor_tensor(
            out=union[:], in0=union[:], in1=inter[:], op=Alu.subtract
        )

        # iou > 0.5  <=>  2*inter > union   (division-free)
        cmp2 = sb.tile([64, 18], f32, tag="cmp2")
        nc.vector.scalar_tensor_tensor(
            out=cmp2[:],
            in0=inter[:],
            scalar=2.0,
            in1=union[:],
            op0=Alu.mult,
            op1=Alu.is_gt,
        )

        # argmax over anchors via cross products (iou_a >= iou_b <=>
        # inter_a*union_b >= inter_b*union_a); first-wins tie-breaking
        inter_r = r3(inter)
        union_r = r3(union)

        def pairprod(name, ia, ib):
            t = sb.tile([64, 6], f32, tag=name)
            nc.vector.tensor_tensor(
                out=t[:].rearrange("p (s b) -> p s b", s=3),
                in0=inter_r[:, :, ia, :],
                in1=union_r[:, :, ib, :],
                op=Alu.mult,
            )
            return t

        p01 = pairprod("p01", 0, 1)
        p10 = pairprod("p10", 1, 0)
        p02 = pairprod("p02", 0, 2)
        p20 = pairprod("p20", 2, 0)
        p12 = pairprod("p12", 1, 2)
        p21 = pairprod("p21", 2, 1)
        ge01 = sb.tile([64, 6], f32, tag="ge01")
        ge02 = sb.tile([64, 6], f32, tag="ge02")
        ge12 = sb.tile([64, 6], f32, tag="ge12")
        nc.vector.tensor_tensor(out=ge01[:], in0=p01[:], in1=p10[:], op=Alu.is_ge)
        nc.vector.tensor_tensor(out=ge02[:], in0=p02[:], in1=p20[:], op=Alu.is_ge)
        nc.vector.tensor_tensor(out=ge12[:], in0=p12[:], in1=p21[:], op=Alu.is_ge)

        oht = sb.tile([64, 18], f32, tag="oht")
        oht_r = r3(oht)
        # oh0 = ge01 & ge02
        nc.vector.tensor_tensor(
            out=oht_r[:, :, 0, :],
            in0=ge01[:].rearrange("p (s b) -> p s b", s=3),
            in1=ge02[:].rearrange("p (s b) -> p s b", s=3),
            op=Alu.mult,
        )
        # oh1 = (1 - ge01) & ge12
        n01 = sb.tile([64, 6], f32, tag="n01")
        nc.vector.tensor_scalar(
            out=n01[:],
            in0=ge01[:],
            scalar1=-1.0,
            scalar2=1.0,
            op0=Alu.mult,
            op1=Alu.add,
        )
        nc.vector.tensor_tensor(
            out=oht_r[:, :, 1, :],
            in0=n01[:].rearrange("p (s b) -> p s b", s=3),
            in1=ge12[:].rearrange("p (s b) -> p s b", s=3),
            op=Alu.mult,
        )
        # oh2 = 1 - oh0 - oh1  (oh0, oh1 mutually exclusive)
        s01 = sb.tile([64, 6], f32, tag="s01")
        nc.vector.tensor_tensor(
            out=s01[:].rearrange("p (s b) -> p s b", s=3),
            in0=oht_r[:, :, 0, :],
            in1=oht_r[:, :, 1, :],
            op=Alu.add,
        )
        nc.vector.tensor_scalar(
            out=oht_r[:, :, 2, :],
            in0=s01[:].rearrange("p (s b) -> p s b", s=3),
            scalar1=-1.0,
            scalar2=1.0,
            op0=Alu.mult,
            op1=Alu.add,
        )

        # w4 = onehot(best anchor) & (iou > 0.5)
        w4 = sb.tile([64, 18], f32, tag="w4")
        nc.vector.tensor_tensor(out=w4[:], in0=oht[:], in1=cmp2[:], op=Alu.mult)

        # ---------- Mja = one-hot(j) replicated per anchor, weighted ----------
        mja = []  # [s][b] -> [64, 3*g] tile, cols (a, h)
        for s, g in enumerate(GS):
            row = []
            for b in range(BL):
                t = sb.tile([64, 3 * g], f32, tag=f"mja{s}_{b}")
                mj_sb = m4[s][:, (2 + b) * g : (3 + b) * g][:, None, :].broadcast_to(
                    [64, 3, g]
                )
                wv = r3(w4)[:, s, :, b][:, :, None].broadcast_to([64, 3, g])
                nc.vector.tensor_tensor(
                    out=t[:].rearrange("p (a g) -> p a g", a=3),
                    in0=mj_sb,
                    in1=wv,
                    op=Alu.mult,
                )
                row.append(t)
            mja.append(row)

        # ---------- per-chunk: gt matmul, BCE ----------
        acc = sb.tile([128, NT], f32, tag="acc")
        nc.vector.memset(acc[:], 0.0)

        for _, k in ISSUE_PLAN:  # process in (approx) arrival order
            s, b, r0, n = CHUNKS[k]
            g = GS[s]
            J = J_BY_SCALE[s]
            P = n // J
            pr_ap = pred_tiles[k][:]

            # gt counts: J matmuls; matmul j covers rows p*J+j -> psum
            # [P, g], binarized into the matching (j, w) slab of gtb
            gtb = sb.tile([P, J * g], f32, tag=f"gtb{k}")
            for j in range(J):
                pt = ps.tile([P, g], f32, tag="gt")
                wj = (
                    mja[s][b][:]
                    .rearrange("t (p j) -> t j p", j=J)[:, j, :]
                )
                nc.tensor.matmul(
                    pt[:],
                    wj,
                    m4[s][:, b * g : (b + 1) * g],
                    start=True,
                    stop=True,
                )
                # binarize gt counts (sole op waiting on PE)
                nc.vector.tensor_scalar(
                    out=gtb[:, j * g : (j + 1) * g],
                    in0=pt[:],
                    scalar1=0.5,
                    scalar2=None,
                    op0=Alu.is_ge,
                )

            # BCE pieces: L1 = ln(1-p), L2 = ln(p)
            l1 = sb.tile([P, J * g], f32, tag=f"l1_{k}")
            l2 = sb.tile([P, J * g], f32, tag=f"l2_{k}")
            dd = sb.tile([P, J * g], f32, tag=f"dd{k}")
            mm = sb.tile([P, J * g], f32, tag=f"mm{k}")
            ee = sb.tile([P, J * g], f32, tag=f"ee{k}")
            nc.scalar.activation(
                out=l1[:],
                in_=pr_ap,
                func=Act.Ln,
                bias=consts[0:P, ONE_OFF : ONE_OFF + 1],
                scale=-1.0,
            )
            nc.scalar.activation(
                out=l2[:],
                in_=pr_ap,
                func=Act.Ln,
                bias=consts[0:P, ZERO_OFF : ZERO_OFF + 1],
            )
            # e = L1 + gtb*(L2-L1); acc[:,k] = sum(e) = -(chunk BCE sum)
            nc.vector.tensor_tensor(out=dd[:], in0=l2[:], in1=l1[:], op=Alu.subtract)
            nc.vector.tensor_tensor(out=mm[:], in0=gtb[:], in1=dd[:], op=Alu.mult)
            nc.vector.scalar_tensor_tensor(
                out=ee[:],
                in0=mm[:],
                scalar=0.0,
                in1=l1[:],
                op0=Alu.bypass,
                op1=Alu.add,
                accum_out=acc[0:P, k : k + 1],
            )

        # raw partials out; host reduces across partitions/cores
        nc.gpsimd.dma_start(out=d_part[:], in_=acc[:])

    _fixup_tail_drain(nc, mybir)
    _BUILT = nc
    return nc


def _fixup_tail_drain(nc, mybir, out_name="partial"):
    """The kernel-tail drain waits on every outstanding semaphore lane, but
    the ISA allows one sync wait per instruction and this walrus refuses to
    split them.  In this kernel every instruction's effect funnels into the
    final 'partial' output DMA (all DMAs and compute feed it transitively),
    so waiting on that DMA's completion semaphore alone is sufficient."""
    fn = nc.m.functions[0]
    out_sem = None
    for blk in fn.blocks:
        for inst in blk.instructions:
            if type(inst).__name__ == "InstDMACopy":
                outs = inst.outs
                if outs and out_name in str(outs[0]):
                    si = inst.sync_info
                    if si is not None and si.on_update:
                        out_sem = si.on_update[0].id
    assert out_sem is not None, "no output DMA with sem update found"
    for blk in fn.blocks:
        for inst in blk.instructions:
            si = inst.sync_info
            if (
                type(inst).__name__ == "InstDrain"
                and si is not None
                and len(si.on_wait) > 1
            ):
                keep = [w for w in si.on_wait if w.id == out_sem]
                assert len(keep) == 1, (
                    f"tail drain: expected exactly one wait on sem {out_sem}, "
                    f"got {[w.id for w in si.on_wait]}"
                )
                inst.sync_info = mybir.SyncInfo(
                    on_wait=keep, on_update=list(si.on_update)
                )


def _make_in_maps(out0, out1, out2, anchors0, anchors1, anchors2, targets):
    base = _const_base()
    anc_flat = np.concatenate(
        [np.asarray(a, np.float32).reshape(-1) for a in (anchors0, anchors1, anchors2)]
    )  # (s, a, d) = 18
    outs = (out0, out1, out2)
    in_maps = []
    for c in range(NCORES):
        sl = slice(c * BL, (c + 1) * BL)
        consts = base.copy()
        consts[:, ANC_OFF : ANC_OFF + 18] = anc_flat[None, :]
        # targets block: rows = t, cols = (b, k)
        tloc = np.asarray(targets[sl], np.float32)  # [BL, T, 5]
        consts[0:T, TGT_OFF : TGT_OFF + 10] = tloc.transpose(1, 0, 2).reshape(T, -1)
        m = {"consts": consts}
        for s in range(3):
            m[f"out{s}"] = np.ascontiguousarray(outs[s][sl])
        in_maps.append(m)
    return in_maps


def _reduce_partials(partials):
    """partials: list of [128, NT] arrays -> scalar loss (float64 accum)."""
    tot = np.zeros(NT, np.float64)
    for p in partials:
        tot += np.asarray(p, np.float64).reshape(-1, NT).sum(axis=0)
    loss = 0.0
    for k, (s, b, r0, n) in enumerate(CHUNKS):
        g = GS[s]
        denom = B * A * g * g
        loss += -tot[k] / denom
    return np.float32(loss)


def _run_hw(in_maps, trace=False):
    from concourse.bass_utils import run_bass_kernel_spmd

    nc = _build()
    br = run_bass_kernel_spmd(nc, in_maps, list(range(NCORES)), trace=trace)
    return br


def kernel(out0, out1, out2, anchors0, anchors1, anchors2, targets):
    in_maps = _make_in_maps(
        out0, out1, out2, anchors0, anchors1, anchors2, targets
    )
    br = _run_hw(in_maps, trace=False)
    partials = [r["partial"] for r in br.results]
    return np.asarray(_reduce_partials(partials), dtype=np.float32)
